# revision 4
# baseline (speedup 1.0000x reference)
"""Trainium2 Bass kernel for nn_MultiHeadAttention (B=2, N=4096, D=512, H=8).

Sharding: 8 cores = 2 batch groups x 4 head-pair shards.
Core c: batch b = c//4, head pair hp = c%4 (global heads 2hp, 2hp+1),
ReduceScatter rank = c%4 within its batch group.

Per-core device pipeline (all shapes per core):
  - projections: qpT/kpT [128(2 heads x 64), 4096] (f32r), vp [4096, 128] (bf16)
  - per head, per 128-row n-chunk:
      scores[n,m] = kp[n]·qp[m] (f32r matmuls, scale pre-folded into w_q)
      exp on ScalarE (accum_out -> softmax denominators), bf16
      normalize via per-partition reciprocal (VectorE) -> bf16 attn row-block
      attn written to DRAM via SWDGE cast-DMA bf16->f32
      PE-transpose row-block -> feat matmuls (featT accumulation in PSUM)
  - w_fc projection partial -> ReduceScatter(add) over the 4-core batch group
  - + residual q, LayerNorm, write out rows [rank*1024:(rank+1)*1024]
Returns (out [2,4096,512], attn [2,8,4096,4096]) matching the reference.
"""

import os
import sys
import types

import numpy as np

# NTFF profile hook shim: this image's antenv lacks axon_hooks; bass_utils
# imports it unconditionally when trace=True under axon.
try:
    from trn_agent_boot.trn_boot import _ntff_profile_via_ctypes

    _hook = _ntff_profile_via_ctypes("/opt/axon/libaxon_pjrt.so")
    _mod = types.ModuleType("antenv.axon_hooks")
    _mod.get_axon_ntff_profile_hook = lambda: _hook
    _mod.set_axon_ntff_profile_hook = lambda h: None
    sys.modules.setdefault("antenv.axon_hooks", _mod)
except Exception:
    pass

import concourse.bacc as bacc
import concourse.mybir as mybir
import concourse.tile as tile
from concourse.bass_utils import run_bass_kernel_spmd
from concourse.masks import make_identity

F32 = mybir.dt.float32
F32R = mybir.dt.float32r
BF16 = mybir.dt.bfloat16

B, N, D, H = 2, 4096, 512, 8
DK = D // H  # 64
NCORES = 8
GROUPS = [[0, 1, 2, 3], [4, 5, 6, 7]]
NSLICE = N // 4  # 1024 rows of `out` per core
EPS = 1e-5

P = 128          # partitions
NCHUNKS = N // P  # 32 attention row chunks
MB = 1024        # m-block width for scores psum / exp calls
ACT_FRAC = 2     # of every 8 transpose-evicts, this many go to ScalarE

last_exec_time_ns = None
_cached = None


def _build():
    nc = bacc.Bacc(None, target_bir_lowering=False)

    qT = nc.dram_tensor("qT", [D, N], F32, kind="ExternalInput")
    kT = nc.dram_tensor("kT", [D, N], F32, kind="ExternalInput")
    vT = nc.dram_tensor("vT", [D, N], F32, kind="ExternalInput")
    q_res = nc.dram_tensor("q_res", [NSLICE, D], F32, kind="ExternalInput")
    wqT = nc.dram_tensor("wqT", [D, P], F32, kind="ExternalInput")
    wkT = nc.dram_tensor("wkT", [D, P], F32, kind="ExternalInput")
    wvT = nc.dram_tensor("wvT", [D, P], F32, kind="ExternalInput")
    wfcT = nc.dram_tensor("wfcT", [P, D], F32, kind="ExternalInput")
    gamma = nc.dram_tensor("gamma", [D], F32, kind="ExternalInput")
    beta = nc.dram_tensor("beta", [D], F32, kind="ExternalInput")

    attn_out = nc.dram_tensor("attn_part", [2, N, N], F32, kind="ExternalOutput")
    out_part = nc.dram_tensor("out_part", [NSLICE, D], F32, kind="ExternalOutput")

    import concourse.bass as bass

    def bcast_row(dram_ap, parts, width):
        # [width] DRAM vector -> [parts, width] AP with 0-stride partitions
        return bass.AP(
            tensor=dram_ap.tensor,
            offset=dram_ap.offset,
            ap=[[0, parts]] + list(dram_ap.ap),
        )

    with tile.TileContext(nc) as tc:
        with (
            tc.tile_pool(name="const", bufs=1) as const,
            tc.tile_pool(name="persist", bufs=1) as persist,
            tc.tile_pool(name="xt", bufs=4) as xt_pool,
            tc.tile_pool(name="ework", bufs=2) as ework,
            tc.tile_pool(name="etw", bufs=3) as etw,
            tc.tile_pool(name="small", bufs=3) as small,
            tc.tile_pool(name="cstage", bufs=3) as cstage,
            tc.tile_pool(name="sc", bufs=2, space="PSUM") as sc_pool,
            tc.tile_pool(name="pt", bufs=3, space="PSUM") as pt_pool,
            tc.tile_pool(name="sm", bufs=1, space="PSUM") as sm_pool,
            tc.tile_pool(name="dram", bufs=1, space="DRAM") as dram,
        ):
            # ---- constants ----
            ident = const.tile([P, P], BF16)
            make_identity(nc, ident)

            wq_sb = const.tile([P, 4, P], F32)
            wk_sb = const.tile([P, 4, P], F32)
            wv_sb = const.tile([P, 4, P], F32)
            for w_dram, w_sb in ((wqT, wq_sb), (wkT, wk_sb), (wvT, wv_sb)):
                for dc in range(4):
                    nc.sync.dma_start(
                        out=w_sb[:, dc, :], in_=w_dram[dc * P : (dc + 1) * P, :]
                    )
            wfc_f32 = const.tile([P, D], F32)
            nc.sync.dma_start(out=wfc_f32[:], in_=wfcT[:])
            wfc_bf = const.tile([P, D], BF16)
            nc.scalar.activation(
                out=wfc_bf[:], in_=wfc_f32[:], func=mybir.ActivationFunctionType.Copy
            )
            gamma_bc = const.tile([P, D], F32)
            beta_bc = const.tile([P, D], F32)
            nc.gpsimd.dma_start(out=gamma_bc[:], in_=bcast_row(gamma[:], P, D))
            nc.gpsimd.dma_start(out=beta_bc[:], in_=bcast_row(beta[:], P, D))
            eps_sb = const.tile([P, 1], F32)
            nc.vector.memset(eps_sb[:], EPS)

            # ---- persistent projections ----
            qpT = persist.tile([P, N], F32R)   # [2*64 head dims, m]
            kpT = persist.tile([P, N], F32R)   # [2*64 head dims, n]
            vp = persist.tile([P, NCHUNKS, P], BF16)  # [m-part, m-chunk, d(2 heads)]
            featT = persist.tile([P, N], BF16)  # [d(2 heads), n]

            # ---- stage A: v projection (vp natural), then qpT/kpT ----
            vt_tiles = [xt_pool.tile([P, N], F32, name=f"vt{dc}", tag="xt") for dc in range(4)]
            for dc in range(4):
                nc.sync.dma_start(out=vt_tiles[dc][:], in_=vT[dc * P : (dc + 1) * P, :])
            for mc in range(NCHUNKS):
                vp_ps = sm_pool.tile([P, P], F32, tag="sm")
                for dc in range(4):
                    nc.tensor.matmul(
                        vp_ps[:],
                        vt_tiles[dc][:, mc * P : (mc + 1) * P],
                        wv_sb[:, dc, :],
                        start=(dc == 0),
                        stop=(dc == 3),
                    )
                nc.scalar.activation(
                    out=vp[:, mc, :],
                    in_=vp_ps[:],
                    func=mybir.ActivationFunctionType.Copy,
                )

            for x_dram, w_sb, out_sb in ((qT, wq_sb, qpT), (kT, wk_sb, kpT)):
                x_tiles = [xt_pool.tile([P, N], F32, name=f"xt{dc}", tag="xt") for dc in range(4)]
                for dc in range(4):
                    nc.sync.dma_start(
                        out=x_tiles[dc][:], in_=x_dram[dc * P : (dc + 1) * P, :]
                    )
                for nq in range(4):
                    pp = sc_pool.tile([P, MB], F32, tag="sc")
                    for dc in range(4):
                        for half in range(2):
                            nc.tensor.matmul(
                                pp[:, half * 512 : (half + 1) * 512],
                                w_sb[:, dc, :],
                                x_tiles[dc][
                                    :,
                                    nq * MB + half * 512 : nq * MB + (half + 1) * 512,
                                ],
                                start=(dc == 0),
                                stop=(dc == 3),
                            )
                    nc.scalar.activation(
                        out=out_sb[:, nq * MB : (nq + 1) * MB],
                        in_=pp[:],
                        func=mybir.ActivationFunctionType.Copy,
                    )

            # ---- stage B: attention ----
            for h in range(2):
                qpT_h = qpT[h * DK : (h + 1) * DK, :]
                kpT_h = kpT[h * DK : (h + 1) * DK, :]
                for ci in range(NCHUNKS):
                    n0 = ci * P
                    # scores + exp (+ per-m-block denom partials)
                    e_raw = ework.tile([P, N], BF16, tag="eraw")
                    den4 = small.tile([P, 4], F32, tag="den4")
                    for mb in range(N // MB):
                        sp = sc_pool.tile([P, MB], F32, tag="sc")
                        for half in range(2):
                            nc.tensor.matmul(
                                sp[:, half * 512 : (half + 1) * 512],
                                kpT_h[:, n0 : n0 + P],
                                qpT_h[:, mb * MB + half * 512 : mb * MB + (half + 1) * 512],
                                start=True,
                                stop=True,
                            )
                        nc.scalar.activation(
                            out=e_raw[:, mb * MB : (mb + 1) * MB],
                            in_=sp[:],
                            func=mybir.ActivationFunctionType.Exp,
                            accum_out=den4[:, mb : mb + 1],
                        )
                    den = small.tile([P, 1], F32, tag="den")
                    nc.vector.tensor_reduce(
                        out=den[:],
                        in_=den4[:],
                        axis=mybir.AxisListType.X,
                        op=mybir.AluOpType.add,
                    )
                    rden = small.tile([P, 1], F32, tag="rden")
                    nc.vector.reciprocal(out=rden[:], in_=den[:])

                    # normalized bf16 attn row-block
                    a_norm = ework.tile([P, N], BF16, tag="anorm")
                    nc.vector.tensor_scalar_mul(
                        out=a_norm[:], in0=e_raw[:], scalar1=rden[:]
                    )
                    # attn out: SWDGE cast-DMA bf16 -> f32
                    nc.gpsimd.dma_start(
                        out=attn_out[h, n0 : n0 + P, :], in_=a_norm[:]
                    )

                    # transpose + feat accumulation
                    fp = sm_pool.tile([P, P], F32, tag="sm")
                    for g in range(8):
                        ptile = pt_pool.tile([P, 512], BF16, tag="pt")
                        for t in range(4):
                            nc.tensor.transpose(
                                ptile[:, t * P : (t + 1) * P],
                                a_norm[:, g * 512 + t * P : g * 512 + (t + 1) * P],
                                ident[:],
                            )
                        et = etw.tile([P, 512], BF16, tag="et")
                        if g % 8 < ACT_FRAC:
                            nc.scalar.activation(
                                out=et[:],
                                in_=ptile[:],
                                func=mybir.ActivationFunctionType.Copy,
                            )
                        else:
                            nc.vector.tensor_copy(out=et[:], in_=ptile[:])
                        for t in range(4):
                            mc = g * 4 + t
                            nc.tensor.matmul(
                                fp[: DK, :],
                                vp[:, mc, h * DK : (h + 1) * DK],
                                et[:, t * P : (t + 1) * P],
                                start=(mc == 0),
                                stop=(mc == NCHUNKS - 1),
                            )
                    nc.scalar.activation(
                        out=featT[h * DK : (h + 1) * DK, n0 : n0 + P],
                        in_=fp[: DK, :],
                        func=mybir.ActivationFunctionType.Copy,
                    )

            # ---- stage C: output projection + ReduceScatter + Add&Norm ----
            fproj = dram.tile([N, D], F32)
            rs_out = dram.tile([NSLICE, D], F32)
            for ci in range(NCHUNKS):
                pp = sc_pool.tile([P, D], F32, tag="sc")
                nc.tensor.matmul(
                    pp[:], featT[:, ci * P : (ci + 1) * P], wfc_bf[:], start=True, stop=True
                )
                st = cstage.tile([P, D], F32, tag="cst")
                nc.scalar.activation(
                    out=st[:], in_=pp[:], func=mybir.ActivationFunctionType.Copy
                )
                nc.sync.dma_start(out=fproj[ci * P : (ci + 1) * P, :], in_=st[:])
            nc.gpsimd.collective_compute(
                "ReduceScatter",
                mybir.AluOpType.add,
                replica_groups=GROUPS,
                ins=[fproj[:].opt()],
                outs=[rs_out[:].opt()],
            )
            for ri in range(NSLICE // P):
                xt = cstage.tile([P, D], F32, tag="cx")
                nc.sync.dma_start(out=xt[:], in_=rs_out[ri * P : (ri + 1) * P, :])
                qt = cstage.tile([P, D], F32, tag="cq")
                nc.sync.dma_start(out=qt[:], in_=q_res[ri * P : (ri + 1) * P, :])
                nc.vector.tensor_add(out=xt[:], in0=xt[:], in1=qt[:])
                stats = small.tile([P, 6], F32, tag="bnst")
                nc.vector.bn_stats(out=stats[:], in_=xt[:])
                mv = small.tile([P, 2], F32, tag="bnmv")
                nc.vector.bn_aggr(out=mv[:], in_=stats[:])
                rstd = small.tile([P, 1], F32, tag="rstd")
                nc.scalar.activation(
                    out=rstd[:],
                    in_=mv[:, 1:2],
                    func=mybir.ActivationFunctionType.Sqrt,
                    bias=eps_sb[:],
                )
                nc.vector.reciprocal(out=rstd[:], in_=rstd[:])
                nc.vector.tensor_scalar(
                    out=xt[:],
                    in0=xt[:],
                    scalar1=mv[:, 0:1],
                    scalar2=rstd[:],
                    op0=mybir.AluOpType.subtract,
                    op1=mybir.AluOpType.mult,
                )
                nc.vector.tensor_mul(out=xt[:], in0=xt[:], in1=gamma_bc[:])
                nc.vector.tensor_add(out=xt[:], in0=xt[:], in1=beta_bc[:])
                nc.sync.dma_start(out=out_part[ri * P : (ri + 1) * P, :], in_=xt[:])

    if not nc.is_finalized():
        nc.finalize()
    return nc


def kernel(q, k, v, w_q, w_k, w_v, w_fc, ln_gamma, ln_beta):
    global last_exec_time_ns, _cached
    q = np.asarray(q, dtype=np.float32)
    k = np.asarray(k, dtype=np.float32)
    v = np.asarray(v, dtype=np.float32)
    w_q = np.asarray(w_q, dtype=np.float32)
    w_k = np.asarray(w_k, dtype=np.float32)
    w_v = np.asarray(w_v, dtype=np.float32)
    w_fc = np.asarray(w_fc, dtype=np.float32)
    ln_gamma = np.asarray(ln_gamma, dtype=np.float32)
    ln_beta = np.asarray(ln_beta, dtype=np.float32)

    if _cached is None:
        _cached = _build()
    nc = _cached

    scale = 1.0 / np.sqrt(np.float32(DK))
    in_maps = []
    for c in range(NCORES):
        b, hp = c // 4, c % 4
        rank = c % 4
        in_maps.append(
            {
                "qT": np.ascontiguousarray(q[b].T),
                "kT": np.ascontiguousarray(k[b].T),
                "vT": np.ascontiguousarray(v[b].T),
                "q_res": np.ascontiguousarray(
                    q[b, rank * NSLICE : (rank + 1) * NSLICE]
                ),
                "wqT": np.ascontiguousarray(
                    (w_q[P * hp : P * (hp + 1), :] * scale).T
                ),
                "wkT": np.ascontiguousarray(w_k[P * hp : P * (hp + 1), :].T),
                "wvT": np.ascontiguousarray(w_v[P * hp : P * (hp + 1), :].T),
                "wfcT": np.ascontiguousarray(w_fc[:, P * hp : P * (hp + 1)].T),
                "gamma": ln_gamma,
                "beta": ln_beta,
            }
        )

    trace = os.environ.get("ATTN_TRACE", "0") == "1"
    res = run_bass_kernel_spmd(nc, in_maps, list(range(NCORES)), trace=trace)
    last_exec_time_ns = res.exec_time_ns

    attn = np.empty((B, H, N, N), dtype=np.float32)
    out = np.empty((B, N, D), dtype=np.float32)
    for c in range(NCORES):
        b, hp = c // 4, c % 4
        rank = c % 4
        attn[b, 2 * hp : 2 * hp + 2] = res.results[c]["attn_part"]
        out[b, rank * NSLICE : (rank + 1) * NSLICE] = res.results[c]["out_part"]
    return out, attn
